# revision 33
# baseline (speedup 1.0000x reference)
"""Bass/Trainium2 kernel for nn_BigramLanguageModel.

Strategy (8 NeuronCores, single SPMD launch, no collectives):
  - The small 3-layer transformer (C=384, T=1024, B=2) is replicated on all
    8 cores (on-chip collectives have a ~10us/ring-step latency floor that
    dwarfs the work they would save).
  - The LM head (C x 50257 GEMM -> 412 MB of logits) dominates compute and
    output bandwidth; it is sharded over the vocab dim: core j computes
    logits[:, 6400*j : 6400*(j+1)] against a padded (384, 51200) Wlm, and
    the host concatenates the shards. Wlm is prefetched under the last MLP.
  - All GEMMs run as float32r (full-rate fp32, ~13-bit mantissa, rel err
    ~1.5e-4 per GEMM). Residual stream h stays fp32 in SBUF all the way.
  - LayerNorm affine (g, b) is folded into the following projection weights
    host-side; 1/sqrt(HS) is folded into Wk; q/k biases are folded into the
    PSUM evacuations (per-partition scalar_tensor_tensor), b1 into the Relu
    evacuation; v/o/mlp2 biases ride as K=1 rank-1 PSUM-preload matmuls.
  - Softmax over the *query* axis (dim=-2 in the reference) is computed in
    the transposed score layout wT[k, t] so the reduction runs along the
    free axis: scores for one (head, key-chunk) land in a (128, 1024)
    two-bank PSUM tile and a single Exp activation with fused accum_out
    yields p and the denominator in one pass over the causally-valid range;
    1/denom is folded into v rows (per-partition scale).
  - The causal mask is a bf16 (-80)-triangle PSUM-preload matmul on the
    diagonal score block (bf16 avoids the fp32r small-N matmul penalty).
  - Head pairs share 128-partition tiles; attention p@v accumulates in
    (64, 1024) PSUM tiles with the two heads in separate banks.
  - PSUM pools are phase-scoped (max 8 banks at any point); evacuations are
    split DVE/ACT by phase load.
"""

import sys

sys.path.insert(0, "/opt/trn_rl_repo")

import numpy as np

import concourse.bass as bass
import concourse.mybir as mybir
import concourse.tile as tile
from concourse import bacc
from concourse import bass_utils

F32 = mybir.dt.float32
BF16 = mybir.dt.bfloat16
F32R = mybir.dt.float32r
I32 = mybir.dt.int32
AF = mybir.ActivationFunctionType
ALU = mybir.AluOpType

V, C, T, H, HS, NL, B = 50257, 384, 1024, 6, 64, 3, 2
P = 128
N = B * T                  # 2048 tokens
NT = N // P                # 16 token chunks
NC3 = C // P               # 3 channel chunks
NCORE = 8
VPAD = 51200               # padded vocab (8 * 6400)
VSH = VPAD // NCORE        # 6400 vocab columns per core
KC = T // P                # 8 key chunks per batch
TB = T // 512              # 2 query blocks of 512 per batch
NEG = -80.0                # mask bias (exp(-80) ~ 1.8e-35)

_CACHE: dict = {}
PHASES: list = []


def _mark(nc, label):
    PHASES.append((label, int(nc.next_id())))


def _valid_lo(kc, tb):
    """First valid query column (within the 512-wide block tb) for key chunk
    kc, or None if the whole block is masked. Valid means t >= 128*kc."""
    lo = 128 * kc - 512 * tb
    if lo >= 512:
        return None
    return max(lo, 0)


class _EvacSplit:
    """Weighted round-robin of PSUM->SBUF evacuation copies over DVE/ACT."""

    def __init__(self, nc, dve_share=2, act_share=1):
        self.nc = nc
        self.i = 0
        self.period = dve_share + act_share
        self.dve_share = dve_share

    def copy(self, out, in_):
        self.i += 1
        if self.i % self.period < self.dve_share:
            self.nc.vector.tensor_copy(out, in_)
        else:
            self.nc.scalar.copy(out, in_)

    def copy_bias(self, out, in_, bias_col):
        """out = in_ + bias (bias per partition, (p,1) AP)."""
        self.i += 1
        if self.i % self.period < self.dve_share:
            self.nc.vector.scalar_tensor_tensor(
                out, in_, bias_col, out,
                op0=mybir.AluOpType.add, op1=mybir.AluOpType.bypass)
        else:
            self.nc.scalar.activation(out, in_, AF.Identity, bias=bias_col)


def _build(has_blm: bool):
    nc = bacc.Bacc("TRN2", target_bir_lowering=False, debug=False)

    # ---------------- DRAM I/O ----------------
    d_idx = nc.dram_tensor("idx", [N, 1], I32, kind="ExternalInput").ap()
    d_tok = nc.dram_tensor("tok_emb", [V, C], BF16, kind="ExternalInput").ap()
    d_pos = nc.dram_tensor("pos", [N, C], BF16, kind="ExternalInput").ap()
    d_wq = nc.dram_tensor("wq", [NL * C, C], BF16, kind="ExternalInput").ap()
    d_wk = nc.dram_tensor("wk", [NL * C, C], BF16, kind="ExternalInput").ap()
    d_wv = nc.dram_tensor("wv", [NL * C, C], BF16, kind="ExternalInput").ap()
    d_wo = nc.dram_tensor("wo", [NL * C, C], BF16, kind="ExternalInput").ap()
    d_w1 = nc.dram_tensor("w1", [NL * C, C], BF16, kind="ExternalInput").ap()
    d_w2 = nc.dram_tensor("w2", [NL * C, C], BF16, kind="ExternalInput").ap()
    d_bv = nc.dram_tensor("bv", [NL, C], BF16, kind="ExternalInput").ap()
    d_bo = nc.dram_tensor("bo", [NL, C], BF16, kind="ExternalInput").ap()
    d_b2 = nc.dram_tensor("b2", [NL, C], BF16, kind="ExternalInput").ap()
    d_ones = nc.dram_tensor("ones", [1, 512], BF16, kind="ExternalInput").ap()
    d_ident = nc.dram_tensor("ident", [P, P], F32, kind="ExternalInput").ap()
    d_identb = nc.dram_tensor("identb", [P, P], BF16, kind="ExternalInput").ap()
    d_trib = nc.dram_tensor("trib", [P, P], BF16, kind="ExternalInput").ap()
    d_bqkt = nc.dram_tensor("bqkt", [NL * C, 2], F32, kind="ExternalInput").ap()
    d_b1t = nc.dram_tensor("b1t", [NL * C, 1], F32, kind="ExternalInput").ap()
    d_wlm = nc.dram_tensor("wlm", [C, VSH], BF16, kind="ExternalInput").ap()
    if has_blm:
        d_blm = nc.dram_tensor("blm", [1, VSH], BF16, kind="ExternalInput").ap()
    d_out = nc.dram_tensor("logits", [N, VSH], BF16, kind="ExternalOutput").ap()

    with tile.TileContext(nc) as tc:
        _emit(nc, tc, locals(), has_blm)
    nc.compile()
    return nc


def _emit(nc, tc, d, has_blm):
    from contextlib import ExitStack

    with ExitStack() as ctx:
        ev_lm = None
        hpool = ctx.enter_context(tc.tile_pool(name="hpool", bufs=NT))
        pers = ctx.enter_context(tc.tile_pool(name="pers", bufs=1))
        spool = ctx.enter_context(tc.tile_pool(name="spool", bufs=8))

        ev = _EvacSplit(nc)

        # preload the one act table that covers every function used in this
        # kernel (exp, ln, relu, identity, copy, square); without this the
        # insert_act_table_loads pass ping-pongs natural_log <-> exp tables
        # around every Ln (1.28us per reload on Act)
        from concourse.hw_specs import get_activation_tables
        tabs = list(get_activation_tables(nc.m.arch))
        set_id = tabs.index("natural_log_exp_and_others")
        nc.scalar.add_instruction(mybir.InstLoadActFuncSet(
            name=nc.scalar.bass.get_next_instruction_name(),
            act_func_set_id=set_id, ins=[], outs=[]))

        # ------------- constants -------------
        ones = pers.tile([1, 512], BF16, name="ones", tag="ones")
        ident = pers.tile([P, P], F32, name="ident", tag="ident")
        identb = pers.tile([P, P], BF16, name="identb", tag="identb")
        trib = pers.tile([P, P], BF16, name="trib", tag="trib")
        eps = pers.tile([P, 1], F32, name="eps", tag="eps")
        nc.sync.dma_start(ones[:], d["d_ones"][:])
        nc.sync.dma_start(ident[:], d["d_ident"][:])
        nc.sync.dma_start(identb[:], d["d_identb"][:])
        nc.sync.dma_start(trib[:], d["d_trib"][:])
        nc.vector.memset(eps[:], 1e-5)

        mvpool = ctx.enter_context(tc.tile_pool(name="mvpool", bufs=3))
        atpool = ctx.enter_context(tc.tile_pool(name="atpool", bufs=3))
        apool = ctx.enter_context(tc.tile_pool(name="apool", bufs=9))

        # ------------- embedding gather -------------
        _mark(nc, "embed")
        h = []  # 16 residual-stream tiles (128, 384) fp32, live whole kernel
        with tc.tile_pool(name="epool", bufs=3) as epool, \
             tc.tile_pool(name="pstre", bufs=2, space="PSUM") as pstre:
            ln1 = _LN(nc, eps, identb, spool, mvpool, atpool, apool, pstre,
                      "aT", evac_act=True)
            idx_t = spool.tile([P, NT], I32, name="idx", tag="idx")
            idx_src = bass.AP(tensor=d["d_idx"].tensor, offset=0,
                              ap=[[1, P], [P, NT]])
            nc.sync.dma_start(idx_t[:], idx_src)
            for i in range(NT):
                emb = epool.tile([P, C], BF16, name="emb", tag="emb")
                nc.gpsimd.indirect_dma_start(
                    out=emb[:], out_offset=None, in_=d["d_tok"][:],
                    in_offset=bass.IndirectOffsetOnAxis(ap=idx_t[:, i:i + 1],
                                                        axis=0),
                )
                pos_t = epool.tile([P, C], BF16, name="pos", tag="pos")
                nc.sync.dma_start(pos_t[:], d["d_pos"][i * P:(i + 1) * P, :])
                h_i = hpool.tile([P, C], F32, name="h", tag="h")
                nc.vector.tensor_add(h_i[:], emb[:], pos_t[:])
                ln1.feed(i, h_i)
                h.append(h_i)
            ln1.finish()

        # ------------- layers -------------
        for l in range(NL):
            with ExitStack() as lctx:
                wpool = lctx.enter_context(
                    tc.tile_pool(name=f"wpool{l}", bufs=1))
                wq = [wpool.tile([P, C], BF16, name=f"wq{c}", tag=f"wq{c}")
                      for c in range(NC3)]
                wk = [wpool.tile([P, C], BF16, name=f"wk{c}", tag=f"wk{c}")
                      for c in range(NC3)]
                wv = [wpool.tile([P, C], BF16, name=f"wv{c}", tag=f"wv{c}")
                      for c in range(NC3)]
                wo = [wpool.tile([P, C], BF16, name=f"wo{c}", tag=f"wo{c}")
                      for c in range(NC3)]
                w1 = [wpool.tile([P, C], BF16, name=f"w1{c}", tag=f"w1{c}")
                      for c in range(NC3)]
                w2 = [wpool.tile([P, C], BF16, name=f"w2{c}", tag=f"w2{c}")
                      for c in range(NC3)]
                for c in range(NC3):
                    r0 = l * C + c * P
                    nc.sync.dma_start(wq[c][:], d["d_wq"][r0:r0 + P, :])
                    nc.sync.dma_start(wk[c][:], d["d_wk"][r0:r0 + P, :])
                    nc.sync.dma_start(wv[c][:], d["d_wv"][r0:r0 + P, :])
                for c in range(NC3):
                    r0 = l * C + c * P
                    nc.sync.dma_start(wo[c][:], d["d_wo"][r0:r0 + P, :])
                    nc.sync.dma_start(w1[c][:], d["d_w1"][r0:r0 + P, :])
                    nc.sync.dma_start(w2[c][:], d["d_w2"][r0:r0 + P, :])
                bqkt = [wpool.tile([P, 2], F32, name=f"bqkt{c}",
                                   tag=f"bqkt{c}") for c in range(NC3)]
                b1t = [wpool.tile([P, 1], F32, name=f"b1t{c}",
                                  tag=f"b1t{c}") for c in range(NC3)]
                for c in range(NC3):
                    r0 = l * C + c * P
                    nc.sync.dma_start(bqkt[c][:], d["d_bqkt"][r0:r0 + P, :])
                    nc.sync.dma_start(b1t[c][:], d["d_b1t"][r0:r0 + P, :])
                bv = wpool.tile([1, C], BF16, name="bv", tag="bv")
                bo = wpool.tile([1, C], BF16, name="bo", tag="bo")
                b2 = wpool.tile([1, C], BF16, name="b2", tag="b2")
                nc.sync.dma_start(bv[:], d["d_bv"][l:l + 1, :])
                nc.sync.dma_start(bo[:], d["d_bo"][l:l + 1, :])
                nc.sync.dma_start(b2[:], d["d_b2"][l:l + 1, :])

                with ExitStack() as actx:
                    attpool = actx.enter_context(
                        tc.tile_pool(name=f"attpool{l}", bufs=3))
                    attT = [attpool.tile([P, N], BF16, name="attT", tag="attT")
                            for _ in range(NC3)]
                    with ExitStack() as qctx:
                        _mark(nc, f"L{l}.ln1")
                        aT = ln1.aT
                        _mark(nc, f"L{l}.v")

                        vpool = qctx.enter_context(
                            tc.tile_pool(name=f"vpool{l}", bufs=NT))
                        psc = qctx.enter_context(tc.tile_pool(
                            name=f"psc{l}", bufs=3, space="PSUM"))
                        psa = qctx.enter_context(tc.tile_pool(
                            name=f"psa{l}", bufs=2, space="PSUM"))
                        v = {}

                        def ensure_v(it):
                            # emit the v GEMM for token chunk `it` on first
                            # use so it fills PE gaps inside the attention
                            # pipeline instead of a serial up-front phase
                            if it not in v:
                                ps = psc.tile([P, C], F32, name="psc",
                                              tag="psc")
                                nc.tensor.matmul(ps[:], ones[:, :P], bv[:],
                                                 start=True, stop=False)
                                for c in range(NC3):
                                    nc.tensor.matmul(
                                        ps[:], aT[c][:, it * P:(it + 1) * P],
                                        wv[c][:], start=False,
                                        stop=(c == NC3 - 1))
                                v_i = vpool.tile([P, C], BF16, name="v",
                                                 tag="v")
                                nc.vector.tensor_copy(v_i[:], ps[:])
                                v[it] = v_i
                            return v[it]

                        qkpool = qctx.enter_context(
                            tc.tile_pool(name=f"qkpool{l}", bufs=2))
                        ppool = qctx.enter_context(
                            tc.tile_pool(name=f"ppool{l}", bufs=4))
                        vspool = qctx.enter_context(
                            tc.tile_pool(name=f"vspool{l}", bufs=6))
                        _mark(nc, f"L{l}.attn")
                        def make_qk(m, evac_act=False):
                            qT_m = qkpool.tile([P, N], BF16, name="qT",
                                               tag="qT")
                            kT_m = qkpool.tile([P, N], BF16, name="kT",
                                               tag="kT")

                            def steps():
                                for dst, wmat, bcol in (
                                        (kT_m, wk, bqkt[m][:, 1:2]),
                                        (qT_m, wq, bqkt[m][:, 0:1])):
                                    for t4 in range(N // 512):
                                        ps = psc.tile([P, 512], F32,
                                                      name="psc", tag="psc")
                                        for c in range(NC3):
                                            nc.tensor.matmul(
                                                ps[:],
                                                wmat[c][:, m * P:(m + 1) * P],
                                                aT[c][:,
                                                      t4 * 512:(t4 + 1) * 512],
                                                start=(c == 0),
                                                stop=(c == NC3 - 1))
                                        if evac_act:
                                            # m=0 build runs in the Act-idle
                                            # window before the first Exp
                                            nc.scalar.activation(
                                                dst[:,
                                                    t4 * 512:(t4 + 1) * 512],
                                                ps[:], AF.Identity,
                                                bias=bcol)
                                        else:
                                            nc.vector.scalar_tensor_tensor(
                                                dst[:,
                                                    t4 * 512:(t4 + 1) * 512],
                                                ps[:], bcol,
                                                dst[:,
                                                    t4 * 512:(t4 + 1) * 512],
                                                op0=ALU.add, op1=ALU.bypass)
                                        yield
                            return qT_m, kT_m, steps()

                        qk = make_qk(0, evac_act=True)
                        for _ in qk[2]:
                            pass  # first build runs up front
                        for m in range(NC3):
                            qT_m, kT_m, _ = qk
                            nxt = make_qk(m + 1) if m + 1 < NC3 else None
                            _attention_bm(nc, tc, l, 0, m, qT_m, kT_m,
                                          ensure_v, attT, ones, trib, identb,
                                          ppool, vspool, spool, psc, psa,
                                          ev, None)
                            # next m's q/k GEMM steps ride inside b=1's kc
                            # loop so Act never drains between m groups
                            _attention_bm(nc, tc, l, 1, m, qT_m, kT_m,
                                          ensure_v, attT, ones, trib, identb,
                                          ppool, vspool, spool, psc, psa,
                                          ev, nxt[2] if nxt else None)
                            qk = nxt

                    _mark(nc, f"L{l}.proj")
                    # --- proj: h += attT.T @ Wo + bo (feeds ln2) ---
                    with tc.tile_pool(name=f"pso{l}", bufs=2,
                                      space="PSUM") as pso, \
                         tc.tile_pool(name=f"pstrb{l}", bufs=2,
                                      space="PSUM") as pstrb:
                        ln2 = _LN(nc, eps, identb, spool, mvpool, atpool,
                                  apool, pstrb, "aT", evac_act=True)
                        for i in range(NT):
                            ps = pso.tile([P, C], F32, name="pmm", tag="pmm")
                            nc.tensor.matmul(ps[:], ones[:, :P], bo[:],
                                             start=True, stop=False)
                            for c in range(NC3):
                                nc.tensor.matmul(
                                    ps[:], attT[c][:, i * P:(i + 1) * P],
                                    wo[c][:], start=False,
                                    stop=(c == NC3 - 1))
                            nc.vector.tensor_add(h[i][:], h[i][:], ps[:])
                            ln2.feed(i, h[i])
                        ln2.finish()

                # --- LN2 + MLP ---
                _mark(nc, f"L{l}.mlp")
                wlm = blm = None
                if l == NL - 1 and not has_blm:
                    # prefetch the LM-head weights under the last MLP
                    lmpool = lctx.enter_context(
                        tc.tile_pool(name="lmpool", bufs=1))
                    wlm = [lmpool.tile([P, VSH], BF16, name=f"wlm{c}",
                                       tag=f"wlm{c}") for c in range(NC3)]
                    for c in range(NC3):
                        nc.sync.dma_start(wlm[c][:],
                                          d["d_wlm"][c * P:(c + 1) * P, :])
                with ExitStack() as mctx:
                    psm = mctx.enter_context(tc.tile_pool(
                        name=f"psm{l}", bufs=6, space="PSUM"))
                    m1pool = mctx.enter_context(
                        tc.tile_pool(name=f"m1pool{l}", bufs=3))
                    a2T = ln2.aT
                    m1T = [m1pool.tile([P, N], BF16, name="m1T", tag="m1T")
                           for _ in range(NC3)]
                    for cm in range(NC3):
                        for t4 in range(N // 512):
                            ps = psm.tile([P, 512], F32, name="pmm", tag="pmm")
                            for c in range(NC3):
                                nc.tensor.matmul(
                                    ps[:], w1[c][:, cm * P:(cm + 1) * P],
                                    a2T[c][:, t4 * 512:(t4 + 1) * 512],
                                    start=(c == 0), stop=(c == NC3 - 1))
                            nc.scalar.activation(
                                m1T[cm][:, t4 * 512:(t4 + 1) * 512],
                                ps[:], AF.Relu, bias=b1t[cm][:, 0:1])
                    last = l == NL - 1
                    with ExitStack() as nctx:
                        if not last:
                            pstrc = nctx.enter_context(tc.tile_pool(
                                name=f"pstrc{l}", bufs=2, space="PSUM"))
                            ln1 = _LN(nc, eps, identb, spool, mvpool,
                                      atpool, apool, pstrc, "aT",
                                      evac_act=True)
                        for i in range(NT):
                            ps = psm.tile([P, C], F32, name="pmm", tag="pmm")
                            nc.tensor.matmul(ps[:], ones[:, :P], b2[:],
                                             start=True, stop=False)
                            for c in range(NC3):
                                nc.tensor.matmul(
                                    ps[:], m1T[c][:, i * P:(i + 1) * P],
                                    w2[c][:], start=False,
                                    stop=(c == NC3 - 1))
                            nc.vector.tensor_add(h[i][:], h[i][:], ps[:])
                            if not last:
                                ln1.feed(i, h[i])
                        if not last:
                            ln1.finish()

                    if last and not has_blm:
                        _lm_head(nc, tc, d, h, ident, ones, ev, has_blm,
                                 wlm, blm, psm)

        if has_blm:
            with tc.tile_pool(name="lmpool", bufs=1) as lmpool:
                wlm = [lmpool.tile([P, VSH], BF16, name=f"wlm{c}",
                                   tag=f"wlm{c}") for c in range(NC3)]
                for c in range(NC3):
                    nc.sync.dma_start(wlm[c][:],
                                      d["d_wlm"][c * P:(c + 1) * P, :])
                blm = lmpool.tile([1, VSH], BF16, name="blm", tag="blm")
                nc.sync.dma_start(blm[:], d["d_blm"][:])
                _lm_head(nc, tc, d, h, ident, ones, ev, has_blm, wlm, blm)

def _lm_head(nc, tc, d, h, ident, ones, ev, has_blm, wlm, blm,
             shared_psum=None):
    from contextlib import ExitStack
    _mark(nc, "lmhead")
    ev = _EvacSplit(nc, dve_share=1, act_share=1)
    # ------------- LM head -------------
    with ExitStack() as lmctx:
        htpool = lmctx.enter_context(tc.tile_pool(name="htpool", bufs=1))
        opool = lmctx.enter_context(tc.tile_pool(
            name="opool", bufs=1 if has_blm else 2))
        if shared_psum is None:
            pslm = lmctx.enter_context(
                tc.tile_pool(name="pslm", bufs=6, space="PSUM"))
        else:
            pslm = shared_psum
        hTw = htpool.tile([P, NC3 * N], BF16, name="hT",
                          tag="hT")
        hT = [bass.AP(tensor=hTw.tensor,
                      offset=hTw.offset + c * N,
                      ap=[hTw.ap[0], [1, N]])
              for c in range(NC3)]
        for i in range(NT):
            pt = pslm.tile([P, C], F32, name="pmm", tag="pmm")
            for c in range(NC3):
                nc.tensor.transpose(pt[:, c * P:(c + 1) * P],
                                    h[i][:, c * P:(c + 1) * P],
                                    ident[:])
            out_ap = bass.AP(tensor=hTw.tensor,
                             offset=hTw.offset + i * P,
                             ap=[hTw.ap[0], [N, NC3], [1, P]])
            ev.copy(out_ap, pt[:])

        nvb = (VSH + 511) // 512  # 13 vocab banks (last is 256 wide)
        halves = [(0, list(range(0, 6)), 3072),
                  (3072, list(range(6, nvb)), VSH - 3072)]
        for i in range(NT):
            for base, vgs_all, wcols in halves:
                ost = opool.tile([P, 3328], BF16, name="ostage",
                                 tag="ostage")
                for g0 in range(0, len(vgs_all), 4):
                    vgs = vgs_all[g0:g0 + 4]
                    pss = {}
                    for vg in vgs:
                        nw = min(512, VSH - vg * 512)
                        pss[vg] = pslm.tile([P, 512], F32, name="pmm",
                                           tag="pmm")
                        if has_blm:
                            nc.tensor.matmul(
                                pss[vg][:, :nw], ones[:, :P],
                                blm[:, vg * 512:vg * 512 + nw],
                                start=True, stop=False)
                    for c in range(NC3):
                        for vg in vgs:
                            nw = min(512, VSH - vg * 512)
                            nc.tensor.matmul(
                                pss[vg][:, :nw],
                                hT[c][:, i * P:(i + 1) * P],
                                wlm[c][:, vg * 512:vg * 512 + nw],
                                start=(c == 0 and not has_blm),
                                stop=(c == NC3 - 1))
                    for vg in vgs:
                        nw = min(512, VSH - vg * 512)
                        ev.copy(ost[:, vg * 512 - base:vg * 512 - base + nw],
                                pss[vg][:, :nw])
                nc.sync.dma_start(
                    d["d_out"][i * P:(i + 1) * P, base:base + wcols],
                    ost[:, :wcols])




class _LN:
    """Incremental LayerNorm + transpose into channel-major bf16 chunks.

    feed(i, h_i) is called by whatever loop produced chunk i of the
    residual stream, immediately after its update; all stats / normalize /
    transpose work for a group of 4 chunks is emitted as soon as the 4th
    chunk of the group is fed, so downstream GEMMs on early chunks never
    queue behind later producer work on the in-order engines.
    rstd = exp(-0.5*ln(var+eps)) keeps Act on the single preloaded exp/ln
    table (no act-table reloads anywhere in the kernel)."""

    def __init__(self, nc, eps, identb, spool, mvpool, atpool, apool, pstr,
                 tag, evac_act=False, interleave=True):
        # pstr may be None at construction (bound later via .pstr) when the
        # transpose PSUM pool only opens after the producing phase
        self.nc = nc
        self.eps = eps
        self.identb = identb
        self.spool = spool
        self.apool = apool
        self.pstr = pstr
        self.evac_act = evac_act
        self.mv = mvpool.tile([P, 2 * NT], F32, name="mv", tag="mv")
        self.lnv = spool.tile([P, NT], F32, name="lnv", tag="lnv")
        self.rstd = spool.tile([P, NT], F32, name="rstd", tag="rstd")
        self.nmr = spool.tile([P, NT], F32, name="nmr", tag="nmr")
        aTw = atpool.tile([P, NC3 * N], BF16, name=tag, tag=tag)
        self.aTw = aTw
        self.aT = [bass.AP(tensor=aTw.tensor, offset=aTw.offset + c * N,
                           ap=[aTw.ap[0], [1, N]]) for c in range(NC3)]
        self.hs = {}
        self.norm = {}
        self.pending = []
        self.interleave = interleave

    def feed(self, i, h_i):
        nc = self.nc
        st = self.spool.tile([P, 6], F32, name="st", tag="st")
        nc.vector.bn_stats(st[:], h_i[:])
        nc.vector.bn_aggr(self.mv[:, 2 * i:2 * i + 2], st[:])
        self.hs[i] = h_i
        if i % 4 != 3 or not self.interleave:
            return
        self._finalize_group(i - 3)

    def _finalize_group(self, g):
        nc = self.nc
        mv = self.mv
        nc.scalar.activation(self.lnv[:, g:g + 4],
                             mv[:, 2 * g + 1:2 * g + 8:2],
                             AF.Ln, bias=self.eps[:, :1])
        nc.scalar.activation(self.rstd[:, g:g + 4], self.lnv[:, g:g + 4],
                             AF.Exp, scale=-0.5)
        nc.vector.scalar_tensor_tensor(self.nmr[:, g:g + 4],
                                       mv[:, 2 * g:2 * g + 8:2], -1.0,
                                       self.rstd[:, g:g + 4],
                                       op0=ALU.mult, op1=ALU.mult)
        for j in range(g, g + 4):
            a_j = self.apool.tile([P, C], BF16, name="a", tag="a")
            # normalize on Act (idle in the LN windows; DVE is the
            # critical producer queue there): a = h*rstd + (-mean*rstd)
            nc.scalar.activation(a_j[:], self.hs[j][:], AF.Identity,
                                 bias=self.nmr[:, j:j + 1],
                                 scale=self.rstd[:, j:j + 1])
            self.norm[j] = a_j
        # transpose a group LATE so the PE never head-of-line blocks on
        # the DVE normalize chain of the group just produced
        self.pending.append(g)
        if len(self.pending) > 1:
            self._transpose_group(self.pending.pop(0))

    def _transpose_group(self, g):
        nc = self.nc
        for j in range(g, g + 4):
            a_j = self.norm.pop(j)
            pt = self.pstr.tile([P, C], BF16, name="ptr", tag="ptr")
            for c in range(NC3):
                nc.tensor.transpose(pt[:, c * P:(c + 1) * P],
                                    a_j[:, c * P:(c + 1) * P],
                                    self.identb[:])
            out_ap = bass.AP(tensor=self.aTw.tensor,
                             offset=self.aTw.offset + j * P,
                             ap=[self.aTw.ap[0], [N, NC3], [1, P]])
            if self.evac_act:
                nc.scalar.copy(out_ap, pt[:])
            else:
                nc.vector.tensor_copy(out_ap, pt[:])

    def finish(self):
        if not self.interleave:
            for g in range(0, NT, 4):
                self._finalize_group(g)
        while self.pending:
            self._transpose_group(self.pending.pop(0))


def _attention_bm(nc, tc, l, b, m, qT_m, kT_m, ensure_v, attT, ones, trib,
                  identb, ppool, vspool, spool, psc, psa, ev, qk_steps):
    """Scores + query-axis softmax + p@v for batch b, heads (2m, 2m+1).

    Scores for one (head, key-chunk) land in a (128, 1024) two-bank PSUM
    tile so a single Exp (with fused row-sum accum_out) covers the whole
    valid range [128*kc : 1024). The softmax denominator is folded into v
    rows (per-partition scale). att accumulates in (64, 1024) PSUM tiles
    with the two heads in separate banks (free halves)."""
    d0 = spool.tile([P, 16], F32, name="d0", tag="d0")
    dinv = spool.tile([P, 16], F32, name="dinv", tag="dinv")

    # (128, 512) per query block: head hh accumulates on partitions
    # [64*hh, 64*hh+64) so each tile is a single PSUM bank
    att_ps = {tb: psa.tile([P, 512], F32, name="patt", tag="patt")
              for tb in range(TB)}
    pending = []

    for kc in range(KC):
        if qk_steps is not None:
            next(qk_steps, None)
        p_kc = ppool.tile([P, 2 * T], BF16, name="p", tag="p")
        ktok = b * T + kc * P
        lo_kc = 128 * kc
        for hh in range(2):
            pp = psc.tile([P, T], F32, name="psc", tag="psc")
            diag_tb = kc // 4
            dcol = 128 * kc
            nc.tensor.matmul(pp[:, dcol:dcol + P], identb[:], trib[:],
                             start=True, stop=False)
            for tb in range(TB):
                lo = _valid_lo(kc, tb)
                if lo is None:
                    continue
                lo_mm = lo
                nc.tensor.matmul(
                    pp[:, tb * 512 + lo_mm:(tb + 1) * 512],
                    kT_m[64 * hh:64 * hh + 64, ktok:ktok + P],
                    qT_m[64 * hh:64 * hh + 64,
                         b * T + tb * 512 + lo_mm:b * T + (tb + 1) * 512],
                    start=(tb != diag_tb), stop=(tb == TB - 1))
            nc.scalar.activation(
                p_kc[:, hh * T + lo_kc:(hh + 1) * T],
                pp[:, lo_kc:T], AF.Exp,
                accum_out=d0[:, 8 * hh + kc:8 * hh + kc + 1])

        # 1/denominator for both heads (cols kc, 8+kc), then fold into v
        nc.vector.reciprocal(dinv[:, kc::8], d0[:, kc::8])
        vs = vspool.tile([P, P], BF16, name="vs", tag="vs")
        it = (b * T + kc * P) // P
        v_it = ensure_v(it)
        for hh in range(2):
            vslice = v_it[:, m * P + 64 * hh:m * P + 64 * hh + 64]
            nc.vector.scalar_tensor_tensor(
                vs[:, 64 * hh:64 * hh + 64], vslice,
                dinv[:, 8 * hh + kc:8 * hh + kc + 1], vslice,
                op0=ALU.mult, op1=ALU.bypass)
        pending.append((kc, p_kc, vs))
        if len(pending) > 2:
            _emit_att(nc, attT, att_ps, m, b, *pending.pop(0))

    while pending:
        _emit_att(nc, attT, att_ps, m, b, *pending.pop(0))
    nc.vector.tensor_copy(
        attT[m][:, b * T + 512:b * T + 1024],
        att_ps[1][:, :])


# ---------------------------------------------------------------------------
# host side
# ---------------------------------------------------------------------------

def _prep_inputs(inputs):
    import ml_dtypes
    f32 = np.float32
    bf16 = ml_dtypes.bfloat16
    tok_emb = np.asarray(inputs["tok_emb"], f32)
    pos_emb = np.asarray(inputs["pos_emb"], f32)
    x = np.asarray(inputs["x"]).astype(np.int32).reshape(N, 1)

    def fold_qkv(W, bias, g, b_ln, extra=1.0):
        # W: (NL, H, C, HS) -> (NL*C, H*HS), rows scaled by g, * extra
        Wf = np.transpose(np.asarray(W, f32), (0, 2, 1, 3)).reshape(NL, C, C)
        bf = (np.asarray(bias, f32).reshape(NL, C)
              + np.einsum("lc,lcd->ld", np.asarray(b_ln, f32), Wf))
        Wg = Wf * np.asarray(g, f32)[:, :, None]
        return (Wg * extra).reshape(NL * C, C), (bf * extra)

    g1, b1n = inputs["ln1_g"], inputs["ln1_b"]
    g2, b2n = inputs["ln2_g"], inputs["ln2_b"]
    wq, bq = fold_qkv(inputs["Wq"], inputs["bq"], g1, b1n)
    wk, bk = fold_qkv(inputs["Wk"], inputs["bk"], g1, b1n, extra=HS ** -0.5)
    wv, bv = fold_qkv(inputs["Wv"], inputs["bv"], g1, b1n)

    W1 = np.asarray(inputs["W1"], f32)
    w1 = (W1 * np.asarray(g2, f32)[:, :, None])
    b1f = (np.asarray(inputs["b1"], f32)
           + np.einsum("lc,lcd->ld", np.asarray(b2n, f32), W1))

    tri = np.zeros((P, P), f32)
    tri[np.tril_indices(P, -1)] = NEG  # tri[k, t] = NEG where t < k
    import ml_dtypes
    trib = tri.astype(ml_dtypes.bfloat16)
    identb = np.eye(P, dtype=ml_dtypes.bfloat16)

    wlm_pad = np.zeros((C, VPAD), f32)
    wlm_pad[:, :V] = np.asarray(inputs["Wlm"], f32)
    blm_pad = np.zeros((1, VPAD), f32)
    blm_pad[0, :V] = np.asarray(inputs["blm"], f32)
    has_blm = bool(np.any(blm_pad))

    common = {
        "idx": x,
        "tok_emb": tok_emb.astype(bf16),
        "pos": np.tile(pos_emb, (B, 1)).astype(bf16),
        "wq": wq.astype(bf16), "wk": wk.astype(bf16), "wv": wv.astype(bf16),
        "wo": np.asarray(inputs["Wo"], f32).reshape(NL * C, C).astype(bf16),
        "w1": w1.reshape(NL * C, C).astype(bf16),
        "w2": np.asarray(inputs["W2"], f32).reshape(NL * C, C).astype(bf16),
        "bv": bv.astype(bf16),
        "bo": np.asarray(inputs["bo"], f32).astype(bf16),
        "b2": np.asarray(inputs["b2"], f32).astype(bf16),
        "ones": np.ones((1, 512), bf16),
        "ident": np.eye(P, dtype=f32),
        "identb": identb,
        "trib": trib,
        "bqkt": np.stack([bq.reshape(-1), bk.reshape(-1)], axis=1),
        "b1t": b1f.reshape(-1, 1),
    }
    wlm_b = wlm_pad.astype(bf16)
    blm_b = blm_pad.astype(bf16)
    in_maps = []
    for j in range(NCORE):
        im = dict(common)
        im["wlm"] = np.ascontiguousarray(wlm_b[:, j * VSH:(j + 1) * VSH])
        if has_blm:
            im["blm"] = np.ascontiguousarray(blm_b[:, j * VSH:(j + 1) * VSH])
        in_maps.append(im)
    return in_maps, has_blm


def kernel(**inputs):
    in_maps, has_blm = _prep_inputs(inputs)
    key = ("nc", has_blm)
    if key not in _CACHE:
        _CACHE[key] = _build(has_blm)
    nc = _CACHE[key]
    res = bass_utils.run_bass_kernel_spmd(nc, in_maps,
                                          core_ids=list(range(NCORE)))
    logits = np.concatenate(
        [np.asarray(r["logits"]).astype(np.float32) for r in res.results],
        axis=1)
    return logits[:, :V].reshape(B, T, V)


if __name__ == "__main__":
    pass

def _emit_att(nc, attT, att_ps, m, b, kc, p_kc, vs):
    for hh in range(2):
        for tb in range(TB):
            lo = _valid_lo(kc, tb)
            if lo is None:
                continue
            last = (kc == (3 if tb == 0 else KC - 1))
            nc.tensor.matmul(
                att_ps[tb][64 * hh:64 * hh + 64, lo:512],
                vs[:, 64 * hh:64 * hh + 64],
                p_kc[:, hh * T + tb * 512 + lo:hh * T + (tb + 1) * 512],
                start=(kc == 0), stop=last, skip_group_check=True)
    if kc == 3:
        nc.vector.tensor_copy(
            attT[m][:, b * T:b * T + 512],
            att_ps[0][:, :])

